# revision 17
# baseline (speedup 1.0000x reference)
# CapsuleNetwork Trainium2 kernel: 8-core data-parallel (batch 256 -> 32/core).
#
# Per-core pipeline (all in one Tile program):
#   conv1 (fp32, im2col K=25) -> conv2 (bf16, kh-pair K-packing) -> conv3 (bf16)
#   -> primary caps conv (bf16, stride 2, N batched over 8-image octets)
#   -> PE transpose + DMA shuffle into capsule-major layout
#   -> u_hat (DVE/GPSIMD fused multiply-accumulate over pose dim)
#   -> 3-iter dynamic routing (softmax on ACT, reductions on DVE, i-sums on PE)
#   -> label mask + 3-layer decoder (bf16 matmuls) -> (v, recon)
import sys
for p in ('/opt/trn_rl_repo',):
    if p not in sys.path:
        sys.path.insert(0, p)

import numpy as np
import ml_dtypes
from contextlib import ExitStack

import concourse.bass as bass
import concourse.tile as tile
from concourse import bacc, mybir
from concourse.bass_utils import run_bass_kernel_spmd

f32 = mybir.dt.float32
bf16 = mybir.dt.bfloat16
i32 = mybir.dt.int32
AF = mybir.ActivationFunctionType
ALU = mybir.AluOpType
AX = mybir.AxisListType

N_CORES = 8
B = 256
B_LOC = B // N_CORES   # 32
OCT = 8
EPS = 1e-7

_cache = {}


def build_program(b_loc=B_LOC):
    nc = bacc.Bacc("TRN2", target_bir_lowering=False, debug=False)
    n_oct = b_loc // OCT

    # ---------------- DRAM I/O ----------------
    x_d = nc.dram_tensor("x", [b_loc, 40, 40], f32, kind="ExternalInput")
    y_d = nc.dram_tensor("y", [b_loc], i32, kind="ExternalInput")
    w1_d = nc.dram_tensor("w1", [25, 64], f32, kind="ExternalInput")
    b1_d = nc.dram_tensor("b1", [64], f32, kind="ExternalInput")
    w2_d = nc.dram_tensor("w2", [128, 45, 128], bf16, kind="ExternalInput")
    b2_d = nc.dram_tensor("b2", [128], f32, kind="ExternalInput")
    w3_d = nc.dram_tensor("w3", [128, 162, 128], bf16, kind="ExternalInput")
    b3_d = nc.dram_tensor("b3", [256], f32, kind="ExternalInput")
    wp_d = nc.dram_tensor("wp", [2, 41, 128, 2, 2, 128], bf16, kind="ExternalInput")
    bp_d = nc.dram_tensor("bp", [256], f32, kind="ExternalInput")
    wr_d = nc.dram_tensor("wr", [128, 9, 8, 160], bf16, kind="ExternalInput")
    d1w_d = nc.dram_tensor("d1w", [160, 1024], bf16, kind="ExternalInput")
    d1b_d = nc.dram_tensor("d1b", [1024], f32, kind="ExternalInput")
    d2w_d = nc.dram_tensor("d2w", [1024, 2048], bf16, kind="ExternalInput")
    d2b_d = nc.dram_tensor("d2b", [2048], f32, kind="ExternalInput")
    d3w_d = nc.dram_tensor("d3w", [2048, 1600], bf16, kind="ExternalInput")
    d3b_d = nc.dram_tensor("d3b", [1600], f32, kind="ExternalInput")
    v_d = nc.dram_tensor("v_out", [b_loc, 160], f32, kind="ExternalOutput")
    rec_d = nc.dram_tensor("recon", [b_loc, 1600], f32, kind="ExternalOutput")

    with tile.TileContext(nc) as tc, ExitStack() as ctx:
        # outer (whole-kernel) pools
        cst = ctx.enter_context(tc.tile_pool(name="cst", bufs=1))
        vpool = ctx.enter_context(tc.tile_pool(name="vpool", bufs=1))
        # routing-section pools (released before the decoder opens)
        rctx = ctx.enter_context(ExitStack())
        wroute = rctx.enter_context(tc.tile_pool(name="wroute", bufs=1))
        upp = rctx.enter_context(tc.tile_pool(name="upp", bufs=2))
        utp = rctx.enter_context(tc.tile_pool(name="utp", bufs=1))
        uhp = rctx.enter_context(tc.tile_pool(name="uhp", bufs=3))
        rtp = rctx.enter_context(tc.tile_pool(name="rtp", bufs=4))
        sqp = rctx.enter_context(tc.tile_pool(name="sqp", bufs=4))
        rps = rctx.enter_context(tc.tile_pool(name="rps", bufs=1, space="PSUM"))
        # conv-section pools (released after the last primary-caps conv)
        cctx = ctx.enter_context(ExitStack())
        wpool = cctx.enter_context(tc.tile_pool(name="wpool", bufs=1))
        wprim = cctx.enter_context(tc.tile_pool(name="wprim", bufs=6))
        x1p = cctx.enter_context(tc.tile_pool(name="x1p", bufs=2))
        x2p = cctx.enter_context(tc.tile_pool(name="x2p", bufs=3))
        x3p = cctx.enter_context(tc.tile_pool(name="x3p", bufs=3))
        a3p = cctx.enter_context(tc.tile_pool(name="a3p", bufs=2))
        a4p = cctx.enter_context(tc.tile_pool(name="a4p", bufs=2))
        cps = cctx.enter_context(tc.tile_pool(name="cps", bufs=1, space="PSUM"))

        # ---------------- constants ----------------
        it32 = cst.tile([128, 128], i32)
        nc.gpsimd.iota(it32[:], [[-1, 128]], base=0, channel_multiplier=1)
        ident = cst.tile([128, 128], f32)
        nc.vector.tensor_scalar(ident[:], it32[:], 0, None, ALU.is_equal)
        tenth = cst.tile([128, 1], bf16)
        nc.vector.memset(tenth[:], 0.1)
        ones_c = cst.tile([128, 1], bf16)
        nc.vector.memset(ones_c[:], 1.0)
        ones_r = cst.tile([1, 128], f32)
        nc.vector.memset(ones_r[:], 1.0)

        # ---------------- weights to SBUF ----------------
        W1 = wpool.tile([25, 64], f32)
        nc.sync.dma_start(out=W1[:], in_=w1_d.ap())
        B1 = wpool.tile([64, 1], f32)
        nc.sync.dma_start(out=B1[:], in_=b1_d.ap().unsqueeze(1))
        W2 = wpool.tile([128, 45, 128], bf16)
        nc.sync.dma_start(out=W2[:], in_=w2_d.ap())
        B2 = wpool.tile([128, 1], f32)
        nc.sync.dma_start(out=B2[:], in_=b2_d.ap().unsqueeze(1))
        W3 = wpool.tile([128, 162, 128], bf16)
        nc.sync.dma_start(out=W3[:], in_=w3_d.ap())
        B3 = wpool.tile([128, 2], f32)
        nc.sync.dma_start(out=B3[:], in_=b3_d.ap().rearrange("(h p) -> p h", h=2))
        BP = wpool.tile([128, 2], f32)
        nc.sync.dma_start(out=BP[:], in_=bp_d.ap().rearrange("(h p) -> p h", h=2))
        WR = wroute.tile([128, 9, 8, 160], bf16)
        nc.sync.dma_start(out=WR[:], in_=wr_d.ap())

        # capsule-major u tiles: per octet, 9 chunks of [128 caps, 8 pose, 8 img]
        UT = [[utp.tile([128, 8, 8], f32, tag=f"ut_{o}_{ic}", name=f"ut_{o}_{ic}") for ic in range(9)]
              for o in range(n_oct)]
        V_ALL = vpool.tile([b_loc, 160], f32)

        def convs_one_image(b):
            # conv1: im2col [25, 36, 36], fp32 matmul K=25
            X1 = x1p.tile([25, 36, 36], f32)
            for kh in range(5):
                src = bass.AP(x_d, b * 1600 + kh * 40, [[1, 5], [40, 36], [1, 36]])
                nc.sync.dma_start(out=X1[5 * kh:5 * kh + 5], in_=src)
            X2 = x2p.tile([128, 36, 36], bf16)
            for c in range(3):
                P1 = cps.tile([64, 12, 36], f32, tag="c1", bufs=1, name="P1")
                nc.tensor.matmul(P1[:], W1[:], X1[:, 12 * c:12 * c + 12, :],
                                 start=True, stop=True)
                nc.scalar.activation(X2[0:64, 12 * c:12 * c + 12, :], P1[:],
                                     AF.Relu, bias=B1[:])
                # second (row-shifted) eviction builds the kh+1 copy in the
                # upper partitions: shifted[r] = act1[r+1]
                if c == 0:
                    nc.scalar.activation(X2[64:128, 0:11, :], P1[:, 1:12, :],
                                         AF.Relu, bias=B1[:])
                else:
                    nc.scalar.activation(X2[64:128, 12 * c - 1:12 * c + 11, :],
                                         P1[:], AF.Relu, bias=B1[:])
            nc.vector.memset(X2[64:128, 35, :], 0.0)

            # conv2: 45 packed taps, out [128, 28, 28] in two oh-halves
            X3 = x3p.tile([128, 28, 28], bf16)
            for c in range(2):
                P2 = cps.tile([128, 14, 28], f32, tag="c2", bufs=2, name="P2")
                t = 0
                for kw in range(9):
                    for pr in range(5):
                        kh = 2 * pr
                        nc.tensor.matmul(
                            P2[:], W2[:, kw * 5 + pr, :],
                            X2[:, kh + 14 * c: kh + 14 * c + 14, kw:kw + 28],
                            start=(t == 0), stop=(t == 44))
                        t += 1
                nc.scalar.activation(X3[:, 14 * c:14 * c + 14, :], P2[:],
                                     AF.Relu, bias=B2[:])
            return X3

        def conv3_one_image(X3, A3, bl):
            for h in range(2):
                P3 = cps.tile([128, 20, 20], f32, tag="c3", bufs=2, name="P3")
                t = 0
                for kh in range(9):
                    for kw in range(9):
                        nc.tensor.matmul(
                            P3[:], W3[:, (kh * 9 + kw) * 2 + h, :],
                            X3[:, kh:kh + 20, kw:kw + 20],
                            start=(t == 0), stop=(t == 80))
                        t += 1
                nc.scalar.activation(A3[h][:, bl], P3[:], AF.Relu,
                                     bias=B3[:, h:h + 1])

        def prim_and_shuffle(o, A3):
            A4 = [a4p.tile([128, OCT, 36], f32, tag=f"a4_{h}", name=f"a4_{h}") for h in range(2)]
            for h in range(2):
                P4 = cps.tile([128, OCT, 6, 6], f32, tag="c4tp", name="P4")
                t = 0
                for kh in range(9):
                    for kw in range(9):
                        tap = kh * 9 + kw
                        if tap % 2 == 0:
                            WT = wprim.tile([128, 2, 2, 128], bf16, tag="wp",
                                            name="WT")
                            nc.sync.dma_start(out=WT[:],
                                              in_=wp_d.ap()[h, tap // 2])
                        for cin in range(2):
                            nc.tensor.matmul(
                                P4[:], WT[:, tap % 2, cin, :],
                                A3[cin][:, :, kh:kh + 11:2, kw:kw + 11:2],
                                start=(t == 0), stop=(t == 161))
                            t += 1
                nc.scalar.activation(A4[h][:], P4[:], AF.Identity,
                                     bias=BP[:, h:h + 1])
            # transpose each [128, 36] image slice -> UPo[:, 128h:128h+128, bl]
            UPo = upp.tile([36, 256, OCT], f32, tag="up", name="UPo")
            for bl in range(OCT):
                for h in range(2):
                    TP = cps.tile([36, 128], f32, tag="c4tp", name="TP")
                    nc.tensor.transpose(TP[:], A4[h][:, bl, :], ident[:])
                    nc.scalar.activation(
                        UPo[:, 128 * h:128 * h + 128, bl], TP[:], AF.Copy)
            # shuffle into capsule-major chunks
            for ic in range(9):
                for px4 in range(4):
                    px = 4 * ic + px4
                    sap = UPo[px:px + 1, :, :] \
                        .rearrange("p (cg v) b -> p cg (v b)", cg=32)
                    nc.sync.dma_start(
                        out=UT[o][ic][32 * px4:32 * px4 + 32, :, :]
                        .rearrange("c v b -> c (v b)"),
                        in_=sap)

        def squash(SP, out_ap):
            # out = s * n / (1+n2), n = n2 * rsqrt(n2)  (EPS folded; |delta| ~1e-6)
            SQ = sqp.tile([1, 160], f32, tag="sq")
            nc.scalar.square(SQ[:], SP[:])
            N2 = sqp.tile([1, 10], f32, tag="n2")
            nc.vector.tensor_reduce(
                N2[:], SQ[:].rearrange("p (j k) -> p j k", j=10), AX.X, ALU.add)
            # rsqrt via bit hack + 2 Newton steps (DVE only, no ACT table)
            Xr = sqp.tile([1, 10], f32, tag="xr")
            xi = Xr[:].bitcast(i32)
            nc.vector.tensor_scalar(xi, N2[:].bitcast(i32), 1, None,
                                    ALU.arith_shift_right)
            nc.vector.tensor_scalar(xi, xi, 0x5f3759df, -1,
                                    ALU.subtract, ALU.mult)
            T1 = sqp.tile([1, 10], f32, tag="t1")
            for _ in range(2):
                nc.vector.tensor_tensor(T1[:], Xr[:], Xr[:], ALU.mult)
                nc.vector.tensor_tensor(T1[:], T1[:], N2[:], ALU.mult)
                nc.vector.tensor_scalar(T1[:], T1[:], -0.5, 1.5, ALU.mult, ALU.add)
                nc.vector.tensor_tensor(Xr[:], Xr[:], T1[:], ALU.mult)
            # f = n2 * rsqrt(n2) / (1 + n2)
            B1t = sqp.tile([1, 10], f32, tag="b1t")
            nc.vector.tensor_scalar(B1t[:], N2[:], 1.0, None, ALU.add)
            RD = sqp.tile([1, 10], f32, tag="rd")
            nc.vector.reciprocal(RD[:], B1t[:])
            F = sqp.tile([1, 10], f32, tag="f")
            nc.vector.tensor_tensor(F[:], N2[:], Xr[:], ALU.mult)
            nc.vector.tensor_tensor(F[:], F[:], RD[:], ALU.mult)
            nc.vector.tensor_tensor(
                out_ap.rearrange("p (j k) -> p j k", j=10),
                SP[:].rearrange("p (j k) -> p j k", j=10),
                F[:].unsqueeze(2).broadcast_to([1, 10, 16]), ALU.mult)

        def agreement(UH, VT, BL, first):
            import dataclasses
            VBS = rtp.tile([128, 160], f32, tag="vbs", name="VBS")
            vsrc = dataclasses.replace(
                VT[:], ap=[[VT[:].ap[0][0], 1], [0, 128], [1, 160]])
            nc.sync.dma_start(out=VBS[:], in_=vsrc)
            for ic in range(9):
                PRD = rtp.tile([128, 10, 16], f32, tag="prd")
                nc.gpsimd.tensor_tensor(
                    PRD[:], UH[:, ic, :].rearrange("p (j k) -> p j k", j=10),
                    VBS[:].rearrange("p (j k) -> p j k", j=10), ALU.mult)
                if first:
                    nc.vector.tensor_reduce(BL[:, ic, :], PRD[:], AX.X, ALU.add)
                else:
                    DL = rtp.tile([128, 10], f32, tag="dl")
                    nc.vector.tensor_reduce(DL[:], PRD[:], AX.X, ALU.add)
                    nc.vector.tensor_tensor(BL[:, ic, :], BL[:, ic, :], DL[:],
                                            ALU.add)

        def route_one_image(b):
            o, bl = b // OCT, b % OCT
            UH = uhp.tile([128, 9, 160], f32)
            for ic in range(9):
                nc.vector.tensor_scalar(UH[:, ic, :], WR[:, ic, 0, :],
                                        UT[o][ic][:, 0, bl:bl + 1], None, ALU.mult)
                for v in range(1, 8):
                    nc.vector.scalar_tensor_tensor(
                        UH[:, ic, :], WR[:, ic, v, :], UT[o][ic][:, v, bl:bl + 1],
                        UH[:, ic, :], ALU.mult, ALU.add)
            # iter 0: c = 1/10 uniform (stream a bf16 shadow of u_hat)
            UHB = uhp.tile([128, 9, 160], bf16, tag="uhb", name="UHB")
            nc.vector.tensor_copy(UHB[:], UH[:])
            SP = rps.tile([1, 160], f32, tag="s", bufs=2, name="SP")
            for ic in range(9):
                nc.tensor.matmul(SP[:], tenth[:], UHB[:, ic, :],
                                 start=(ic == 0), stop=(ic == 8))
            VT = rtp.tile([1, 160], f32, tag="vt")
            squash(SP, VT[:])
            BL = rtp.tile([128, 9, 10], f32, tag="bl")
            agreement(UH, VT, BL, first=True)
            # iters 1, 2
            for it in (1, 2):
                E = rtp.tile([128, 9, 10], f32, tag="e")
                nc.scalar.activation(E[:], BL[:], AF.Exp)
                SM = rtp.tile([128, 9], f32, tag="sm")
                nc.vector.tensor_reduce(SM[:], E[:], AX.X, ALU.add)
                RS = rtp.tile([128, 9], f32, tag="rs")
                nc.vector.reciprocal(RS[:], SM[:])
                EN = rtp.tile([128, 9, 10], f32, tag="en")
                nc.vector.tensor_tensor(
                    EN[:], E[:], RS[:].unsqueeze(2).broadcast_to([128, 9, 10]),
                    ALU.mult)
                SP2 = rps.tile([1, 160], f32, tag="s", bufs=2, name="SP2")
                for ic in range(9):
                    T = rtp.tile([128, 10, 16], bf16, tag="t")
                    nc.gpsimd.tensor_tensor(
                        T[:], UH[:, ic, :].rearrange("p (j k) -> p j k", j=10),
                        EN[:, ic, :].unsqueeze(2).broadcast_to([128, 10, 16]),
                        ALU.mult)
                    nc.tensor.matmul(SP2[:], ones_c[:],
                                     T[:].rearrange("p j k -> p (j k)"),
                                     start=(ic == 0), stop=(ic == 8))
                VT2 = rtp.tile([1, 160], f32, tag="vt")
                squash(SP2, VT2[:])
                if it == 1:
                    agreement(UH, VT2, BL, first=False)
                else:
                    # engines cannot shift partitions; route via DMA
                    nc.sync.dma_start(out=V_ALL[b:b + 1, :], in_=VT2[:])

        # ---------------- emit: software-pipelined over octets ----------------
        for o in range(n_oct):
            A3 = [a3p.tile([128, OCT, 20, 20], bf16, tag=f"a3_{h}", name=f"a3_{h}")
                  for h in range(2)]
            for bl in range(OCT):
                b = o * OCT + bl
                X3 = convs_one_image(b)
                conv3_one_image(X3, A3, bl)
                if o > 0:
                    route_one_image((o - 1) * OCT + bl)
            prim_and_shuffle(o, A3)
        cctx.close()
        for bl in range(OCT):
            route_one_image((n_oct - 1) * OCT + bl)
        rctx.close()

        # ---------------- mask + decoder ----------------
        decp = ctx.enter_context(tc.tile_pool(name="decp", bufs=1))
        dps = ctx.enter_context(tc.tile_pool(name="dps", bufs=1, space="PSUM"))
        Yt = decp.tile([b_loc, 1], i32)
        nc.sync.dma_start(out=Yt[:], in_=y_d.ap().unsqueeze(1))
        Yf = decp.tile([b_loc, 1], f32)
        nc.vector.tensor_copy(Yf[:], Yt[:])
        JI = decp.tile([b_loc, 160], i32)
        nc.gpsimd.iota(JI[:].rearrange("p (j k) -> p j k", j=10),
                       [[1, 10], [0, 16]], base=0, channel_multiplier=0)
        JF = decp.tile([b_loc, 160], f32)
        nc.vector.tensor_copy(JF[:], JI[:])
        MK = decp.tile([b_loc, 160], f32)
        nc.vector.tensor_tensor(MK[:], JF[:],
                                Yf[:].broadcast_to([b_loc, 160]), ALU.is_equal)
        VM = decp.tile([b_loc, 160], f32)
        nc.vector.tensor_tensor(VM[:], V_ALL[:], MK[:], ALU.mult)
        nc.sync.dma_start(out=v_d.ap(), in_=V_ALL[:])

        # vmT via two PE transposes
        TA = dps.tile([128, b_loc], f32, tag="tp2", name="TA")
        nc.tensor.transpose(TA[:], VM[:, 0:128], ident[0:b_loc, 0:b_loc])
        VTA = decp.tile([128, b_loc], bf16)
        nc.scalar.activation(VTA[:], TA[:], AF.Copy)
        TBp = dps.tile([32, b_loc], f32, tag="tp2", name="TBp")
        nc.tensor.transpose(TBp[:], VM[:, 128:160], ident[0:b_loc, 0:b_loc])
        VTB = decp.tile([32, b_loc], bf16)
        nc.scalar.activation(VTB[:], TBp[:], AF.Copy)

        D1A = decp.tile([128, 1024], bf16)
        nc.sync.dma_start(out=D1A[:], in_=d1w_d.ap()[0:128, :])
        D1Bw = decp.tile([32, 1024], bf16)
        nc.sync.dma_start(out=D1Bw[:], in_=d1w_d.ap()[128:160, :])
        D1b = decp.tile([128, 8], f32)
        nc.sync.dma_start(out=D1b[:], in_=d1b_d.ap().rearrange("(m p) -> p m", m=8))
        D2W = decp.tile([128, 8, 2048], bf16)
        nc.sync.dma_start(out=D2W[:],
                          in_=d2w_d.ap().rearrange("(c p) m -> p c m", p=128))
        D2b = decp.tile([128, 16], f32)
        nc.sync.dma_start(out=D2b[:], in_=d2b_d.ap().rearrange("(m p) -> p m", m=16))
        D3W = decp.tile([128, 16, 1600], bf16)
        nc.sync.dma_start(out=D3W[:],
                          in_=d3w_d.ap().rearrange("(c p) m -> p c m", p=128))
        D3b = decp.tile([128, 13], f32)
        nc.sync.dma_start(out=D3b[:, 0:12],
                          in_=d3b_d.ap()[0:1536].rearrange("(m p) -> p m", m=12))
        nc.sync.dma_start(out=D3b[0:64, 12:13], in_=d3b_d.ap()[1536:1600].unsqueeze(1))

        R1 = decp.tile([128, 8, b_loc], bf16)
        for mc in range(8):
            PS = dps.tile([128, b_loc], f32, tag="d", bufs=2, name="PS")
            nc.tensor.matmul(PS[:], D1A[:, 128 * mc:128 * mc + 128], VTA[:],
                             start=True, stop=False)
            nc.tensor.matmul(PS[:], D1Bw[:, 128 * mc:128 * mc + 128], VTB[:],
                             start=False, stop=True)
            nc.scalar.activation(R1[:, mc, :], PS[:], AF.Relu,
                                 bias=D1b[:, mc:mc + 1])
        R2 = decp.tile([128, 16, b_loc], bf16)
        for mc in range(16):
            PS = dps.tile([128, b_loc], f32, tag="d", bufs=2, name="PS")
            for kc in range(8):
                nc.tensor.matmul(PS[:], D2W[:, kc, 128 * mc:128 * mc + 128],
                                 R1[:, kc, :], start=(kc == 0), stop=(kc == 7))
            nc.scalar.activation(R2[:, mc, :], PS[:], AF.Relu,
                                 bias=D2b[:, mc:mc + 1])
        for mc in range(13):
            mw = 128 if mc < 12 else 64
            PS = dps.tile([128, b_loc], f32, tag="d", bufs=2, name="PS")
            for kc in range(16):
                nc.tensor.matmul(PS[0:mw, :],
                                 D3W[:, kc, 128 * mc:128 * mc + mw],
                                 R2[:, kc, :], start=(kc == 0), stop=(kc == 15))
            R3 = decp.tile([128, b_loc], f32, tag="r3")
            nc.scalar.activation(R3[0:mw, :], PS[0:mw, :], AF.Sigmoid,
                                 bias=D3b[0:mw, mc:mc + 1])
            nc.sync.dma_start(
                out=rec_d.ap()[:, 128 * mc:128 * mc + mw].transpose([1, 0]),
                in_=R3[0:mw, :])

    nc.compile()
    return nc


def prep_inputs(inputs, lo, hi):
    """Host-side prep: shard batch, cast/relayout weights. Returns in_map."""
    g = {k: np.asarray(v) for k, v in inputs.items()}
    bf = ml_dtypes.bfloat16

    w2 = g["conv2_k"].astype(np.float32)          # [9,9,64,128]
    w2p = np.zeros((45, 128, 128), np.float32)
    for kw in range(9):
        for pr in range(5):
            t = kw * 5 + pr
            kh = 2 * pr
            w2p[t, 0:64] = w2[kh, kw]
            if kh + 1 <= 8:
                w2p[t, 64:128] = w2[kh + 1, kw]
    w3 = g["conv3_k"].astype(np.float32)          # [9,9,128,256]
    w3t = np.zeros((162, 128, 128), np.float32)
    for kh in range(9):
        for kw in range(9):
            for h in range(2):
                w3t[(kh * 9 + kw) * 2 + h] = w3[kh, kw, :, 128 * h:128 * h + 128]
    wp = g["prim_k"].astype(np.float32)           # [9,9,256,256]
    # [h, tap_pair, ci_part, (tap%2, cin... )]: device reads [h, t2, p, s, co]
    # where lhsT for (tap, cin, h) = wpt2[h, tap//2, :, tap%2 ... ] -- we pack
    # (tap%2, cin) into the s axis of size 2x2=4? No: tile is [128, 2, 2, 128]
    # (tap-parity, cin). DMA src must match: [h, tap//2, p, (parity, cin)*? ]
    wpt4 = np.zeros((2, 41, 128, 2, 2, 128), np.float32)  # h, t2, p, parity, cin, co
    for kh in range(9):
        for kw in range(9):
            tap = kh * 9 + kw
            for cin in range(2):
                for h in range(2):
                    wpt4[h, tap // 2, :, tap % 2, cin, :] = \
                        wp[kh, kw, 128 * cin:128 * cin + 128, 128 * h:128 * h + 128]
    wr = g["w"].astype(np.float32)                # [1152,10,16,8]
    wrt = wr.transpose(0, 3, 1, 2).reshape(9, 128, 8, 160)

    return dict(
        x=np.ascontiguousarray(g["x"][lo:hi, :, :, 0], dtype=np.float32),
        y=np.ascontiguousarray(g["y"][lo:hi], dtype=np.int32),
        w1=np.ascontiguousarray(g["conv1_k"].reshape(25, 64), dtype=np.float32),
        b1=g["conv1_b"].astype(np.float32),
        w2=np.ascontiguousarray(w2p.transpose(1, 0, 2)).astype(bf),
        b2=g["conv2_b"].astype(np.float32),
        w3=np.ascontiguousarray(w3t.transpose(1, 0, 2)).astype(bf),
        b3=g["conv3_b"].astype(np.float32),
        wp=wpt4.astype(bf),
        bp=g["prim_b"].astype(np.float32),
        wr=np.ascontiguousarray(wrt.transpose(1, 0, 2, 3)).astype(bf),
        d1w=g["d1_w"].astype(bf), d1b=g["d1_b"].astype(np.float32),
        d2w=g["d2_w"].astype(bf), d2b=g["d2_b"].astype(np.float32),
        d3w=g["d3_w"].astype(bf), d3b=g["d3_b"].astype(np.float32),
    )


def kernel(**inputs):
    if "nc" not in _cache:
        _cache["nc"] = build_program()
    nc = _cache["nc"]
    in_maps = [prep_inputs(inputs, c * B_LOC, (c + 1) * B_LOC)
               for c in range(N_CORES)]
    try:
        res = run_bass_kernel_spmd(nc, in_maps, list(range(N_CORES)))
    except Exception:
        # transient NRT device wedges recover on retry
        import time as _time
        _time.sleep(2.0)
        res = run_bass_kernel_spmd(nc, in_maps, list(range(N_CORES)))
    v = np.concatenate([res.results[c]["v_out"].reshape(B_LOC, 1, 10, 16)
                        for c in range(N_CORES)], axis=0)
    recon = np.concatenate([res.results[c]["recon"] for c in range(N_CORES)],
                           axis=0)
    return v.astype(np.float32), recon.astype(np.float32)


# revision 20
# speedup vs baseline: 1.1577x; 1.1577x over previous
# CapsuleNetwork Trainium2 kernel: 8-core data-parallel (batch 256 -> 32/core).
#
# Per-core pipeline (all in one Tile program):
#   conv1 (fp32, im2col K=25) -> conv2 (bf16, kh-pair K-packing) -> conv3 (bf16)
#   -> primary caps conv (bf16, stride 2, N batched over 8-image octets)
#   -> PE transpose + DMA shuffle into capsule-major layout
#   -> u_hat (DVE/GPSIMD fused multiply-accumulate over pose dim)
#   -> 3-iter dynamic routing (softmax on ACT, reductions on DVE, i-sums on PE)
#   -> label mask + 3-layer decoder (bf16 matmuls) -> (v, recon)
import sys
for p in ('/opt/trn_rl_repo',):
    if p not in sys.path:
        sys.path.insert(0, p)

import numpy as np
import ml_dtypes
from contextlib import ExitStack

import concourse.bass as bass
import concourse.tile as tile
from concourse import bacc, mybir
from concourse.bass_utils import run_bass_kernel_spmd

f32 = mybir.dt.float32
bf16 = mybir.dt.bfloat16
i32 = mybir.dt.int32
AF = mybir.ActivationFunctionType
ALU = mybir.AluOpType
AX = mybir.AxisListType

N_CORES = 8
B = 256
B_LOC = B // N_CORES   # 32
OCT = 8
EPS = 1e-7

_cache = {}


def build_program(b_loc=B_LOC):
    nc = bacc.Bacc("TRN2", target_bir_lowering=False, debug=False)
    n_oct = b_loc // OCT

    # ---------------- DRAM I/O ----------------
    x_d = nc.dram_tensor("x", [b_loc, 40, 40], f32, kind="ExternalInput")
    y_d = nc.dram_tensor("y", [b_loc], i32, kind="ExternalInput")
    w1_d = nc.dram_tensor("w1", [25, 64], f32, kind="ExternalInput")
    b1_d = nc.dram_tensor("b1", [64], f32, kind="ExternalInput")
    w2_d = nc.dram_tensor("w2", [128, 45, 128], bf16, kind="ExternalInput")
    b2_d = nc.dram_tensor("b2", [128], f32, kind="ExternalInput")
    w3_d = nc.dram_tensor("w3", [128, 162, 128], bf16, kind="ExternalInput")
    b3_d = nc.dram_tensor("b3", [256], f32, kind="ExternalInput")
    wp_d = nc.dram_tensor("wp", [2, 41, 128, 2, 2, 128], bf16, kind="ExternalInput")
    bp_d = nc.dram_tensor("bp", [256], f32, kind="ExternalInput")
    wr_d = nc.dram_tensor("wr", [128, 9, 8, 160], bf16, kind="ExternalInput")
    d1w_d = nc.dram_tensor("d1w", [160, 1024], bf16, kind="ExternalInput")
    d1b_d = nc.dram_tensor("d1b", [1024], f32, kind="ExternalInput")
    d2w_d = nc.dram_tensor("d2w", [1024, 2048], bf16, kind="ExternalInput")
    d2b_d = nc.dram_tensor("d2b", [2048], f32, kind="ExternalInput")
    d3w_d = nc.dram_tensor("d3w", [2048, 1600], bf16, kind="ExternalInput")
    d3b_d = nc.dram_tensor("d3b", [1600], f32, kind="ExternalInput")
    v_d = nc.dram_tensor("v_out", [b_loc, 160], f32, kind="ExternalOutput")
    rec_d = nc.dram_tensor("recon", [b_loc, 1600], f32, kind="ExternalOutput")

    with tile.TileContext(nc) as tc, ExitStack() as ctx:
        # outer (whole-kernel) pools
        cst = ctx.enter_context(tc.tile_pool(name="cst", bufs=1))
        vpool = ctx.enter_context(tc.tile_pool(name="vpool", bufs=1))
        # routing-section pools (released before the decoder opens)
        rctx = ctx.enter_context(ExitStack())
        wroute = rctx.enter_context(tc.tile_pool(name="wroute", bufs=1))
        upp = rctx.enter_context(tc.tile_pool(name="upp", bufs=2))
        utp = rctx.enter_context(tc.tile_pool(name="utp", bufs=1))
        uhp = rctx.enter_context(tc.tile_pool(name="uhp", bufs=3))
        rtp = rctx.enter_context(tc.tile_pool(name="rtp", bufs=4))
        sqp = rctx.enter_context(tc.tile_pool(name="sqp", bufs=4))
        rps = rctx.enter_context(tc.tile_pool(name="rps", bufs=1, space="PSUM"))
        # conv-section pools (released after the last primary-caps conv)
        cctx = ctx.enter_context(ExitStack())
        wpool = cctx.enter_context(tc.tile_pool(name="wpool", bufs=1))
        wprim = cctx.enter_context(tc.tile_pool(name="wprim", bufs=10))
        x1p = cctx.enter_context(tc.tile_pool(name="x1p", bufs=2))
        x2p = cctx.enter_context(tc.tile_pool(name="x2p", bufs=3))
        x3p = cctx.enter_context(tc.tile_pool(name="x3p", bufs=3))
        a3p = cctx.enter_context(tc.tile_pool(name="a3p", bufs=2))
        a4p = cctx.enter_context(tc.tile_pool(name="a4p", bufs=2))
        cps = cctx.enter_context(tc.tile_pool(name="cps", bufs=1, space="PSUM"))

        # ---------------- constants ----------------
        it32 = cst.tile([128, 128], i32)
        nc.gpsimd.iota(it32[:], [[-1, 128]], base=0, channel_multiplier=1)
        ident = cst.tile([128, 128], f32)
        nc.vector.tensor_scalar(ident[:], it32[:], 0, None, ALU.is_equal)
        tenth = cst.tile([128, 1], bf16)
        nc.vector.memset(tenth[:], 0.1)
        ones_c = cst.tile([128, 1], bf16)
        nc.vector.memset(ones_c[:], 1.0)
        ones_r = cst.tile([1, 128], f32)
        nc.vector.memset(ones_r[:], 1.0)

        # ---------------- weights to SBUF ----------------
        W1 = wpool.tile([25, 64], f32)
        nc.sync.dma_start(out=W1[:], in_=w1_d.ap())
        B1 = wpool.tile([64, 1], f32)
        nc.sync.dma_start(out=B1[:], in_=b1_d.ap().unsqueeze(1))
        W2 = wpool.tile([128, 45, 128], bf16)
        nc.sync.dma_start(out=W2[:], in_=w2_d.ap())
        B2 = wpool.tile([128, 1], f32)
        nc.sync.dma_start(out=B2[:], in_=b2_d.ap().unsqueeze(1))
        W3 = wpool.tile([128, 162, 128], bf16)
        nc.sync.dma_start(out=W3[:], in_=w3_d.ap())
        B3 = wpool.tile([128, 2], f32)
        nc.sync.dma_start(out=B3[:], in_=b3_d.ap().rearrange("(h p) -> p h", h=2))
        BP = wpool.tile([128, 2], f32)
        nc.sync.dma_start(out=BP[:], in_=bp_d.ap().rearrange("(h p) -> p h", h=2))
        WR = wroute.tile([128, 9, 8, 160], bf16)
        nc.sync.dma_start(out=WR[:], in_=wr_d.ap())

        # capsule-major u tiles: per octet, 9 chunks of [128 caps, 8 pose, 8 img]
        UT = [[utp.tile([128, 8, 8], f32, tag=f"ut_{o}_{ic}", name=f"ut_{o}_{ic}") for ic in range(9)]
              for o in range(n_oct)]
        V_ALL = vpool.tile([b_loc, 160], f32)

        def convs_one_image(b):
            # conv1: im2col [25, 36, 36], fp32 matmul K=25
            X1 = x1p.tile([25, 36, 36], f32)
            for kh in range(5):
                src = bass.AP(x_d, b * 1600 + kh * 40, [[1, 5], [40, 36], [1, 36]])
                nc.sync.dma_start(out=X1[5 * kh:5 * kh + 5], in_=src)
            X2 = x2p.tile([128, 36, 36], bf16)
            for c in range(3):
                P1 = cps.tile([64, 12, 36], f32, tag="c1", bufs=1, name="P1")
                nc.tensor.matmul(P1[:], W1[:], X1[:, 12 * c:12 * c + 12, :],
                                 start=True, stop=True)
                nc.scalar.activation(X2[0:64, 12 * c:12 * c + 12, :], P1[:],
                                     AF.Relu, bias=B1[:])
                # second (row-shifted) eviction builds the kh+1 copy in the
                # upper partitions: shifted[r] = act1[r+1]
                if c == 0:
                    nc.scalar.activation(X2[64:128, 0:11, :], P1[:, 1:12, :],
                                         AF.Relu, bias=B1[:])
                else:
                    nc.scalar.activation(X2[64:128, 12 * c - 1:12 * c + 11, :],
                                         P1[:], AF.Relu, bias=B1[:])
            nc.vector.memset(X2[64:128, 35, :], 0.0)

            # conv2: 45 packed taps, out [128, 28, 28] in two oh-halves
            X3 = x3p.tile([128, 28, 28], bf16)
            for c in range(2):
                P2 = cps.tile([128, 14, 28], f32, tag="c2", bufs=2, name="P2")
                t = 0
                for kw in range(9):
                    for pr in range(5):
                        kh = 2 * pr
                        nc.tensor.matmul(
                            P2[:], W2[:, kw * 5 + pr, :],
                            X2[:, kh + 14 * c: kh + 14 * c + 14, kw:kw + 28],
                            start=(t == 0), stop=(t == 44))
                        t += 1
                nc.scalar.activation(X3[:, 14 * c:14 * c + 14, :], P2[:],
                                     AF.Relu, bias=B2[:])
            return X3

        def conv3_one_image(X3, A3, bl):
            for h in range(2):
                P3 = cps.tile([128, 20, 20], f32, tag="c3", bufs=2, name="P3")
                t = 0
                for kh in range(9):
                    for kw in range(9):
                        nc.tensor.matmul(
                            P3[:], W3[:, (kh * 9 + kw) * 2 + h, :],
                            X3[:, kh:kh + 20, kw:kw + 20],
                            start=(t == 0), stop=(t == 80))
                        t += 1
                nc.scalar.activation(A3[h][:, bl], P3[:], AF.Relu,
                                     bias=B3[:, h:h + 1])

        def prim_and_shuffle(o, A3):
            A4 = [a4p.tile([128, OCT, 36], f32, tag=f"a4_{h}", name=f"a4_{h}") for h in range(2)]
            for h in range(2):
                P4 = cps.tile([128, OCT, 6, 6], f32, tag="c4tp", name="P4")
                t = 0
                for kh in range(9):
                    for kw in range(9):
                        tap = kh * 9 + kw
                        if tap % 2 == 0:
                            WT = wprim.tile([128, 2, 2, 128], bf16, tag="wp",
                                            name="WT")
                            nc.sync.dma_start(out=WT[:],
                                              in_=wp_d.ap()[h, tap // 2])
                        for cin in range(2):
                            nc.tensor.matmul(
                                P4[:], WT[:, tap % 2, cin, :],
                                A3[cin][:, :, kh:kh + 11:2, kw:kw + 11:2],
                                start=(t == 0), stop=(t == 161))
                            t += 1
                nc.scalar.activation(A4[h][:], P4[:], AF.Identity,
                                     bias=BP[:, h:h + 1])
            # transpose each [128, 36] image slice -> UPo[:, 128h:128h+128, bl]
            UPo = upp.tile([36, 256, OCT], f32, tag="up", name="UPo")
            for bl in range(OCT):
                for h in range(2):
                    TP = cps.tile([36, 128], f32, tag="c4tp", name="TP")
                    nc.tensor.transpose(TP[:], A4[h][:, bl, :], ident[:])
                    nc.scalar.activation(
                        UPo[:, 128 * h:128 * h + 128, bl], TP[:], AF.Copy)
            # shuffle into capsule-major chunks
            for ic in range(9):
                for px4 in range(4):
                    px = 4 * ic + px4
                    sap = UPo[px:px + 1, :, :] \
                        .rearrange("p (cg v) b -> p cg (v b)", cg=32)
                    nc.sync.dma_start(
                        out=UT[o][ic][32 * px4:32 * px4 + 32, :, :]
                        .rearrange("c v b -> c (v b)"),
                        in_=sap)

        def squash(SP, out_ap):
            # out = s * n / (1+n2), n = n2 * rsqrt(n2)  (EPS folded; |delta| ~1e-6)
            SQ = sqp.tile([1, 160], f32, tag="sq")
            nc.scalar.square(SQ[:], SP[:])
            N2 = sqp.tile([1, 10], f32, tag="n2")
            nc.vector.tensor_reduce(
                N2[:], SQ[:].rearrange("p (j k) -> p j k", j=10), AX.X, ALU.add)
            # rsqrt via bit hack + 2 Newton steps (DVE only, no ACT table)
            Xr = sqp.tile([1, 10], f32, tag="xr")
            xi = Xr[:].bitcast(i32)
            nc.vector.tensor_scalar(xi, N2[:].bitcast(i32), 1, None,
                                    ALU.arith_shift_right)
            nc.vector.tensor_scalar(xi, xi, 0x5f3759df, -1,
                                    ALU.subtract, ALU.mult)
            T1 = sqp.tile([1, 10], f32, tag="t1")
            for _ in range(2):
                nc.vector.tensor_tensor(T1[:], Xr[:], Xr[:], ALU.mult)
                nc.vector.tensor_tensor(T1[:], T1[:], N2[:], ALU.mult)
                nc.vector.tensor_scalar(T1[:], T1[:], -0.5, 1.5, ALU.mult, ALU.add)
                nc.vector.tensor_tensor(Xr[:], Xr[:], T1[:], ALU.mult)
            # f = n2 * rsqrt(n2) / (1 + n2)
            B1t = sqp.tile([1, 10], f32, tag="b1t")
            nc.vector.tensor_scalar(B1t[:], N2[:], 1.0, None, ALU.add)
            RD = sqp.tile([1, 10], f32, tag="rd")
            nc.vector.reciprocal(RD[:], B1t[:])
            F = sqp.tile([1, 10], f32, tag="f")
            nc.vector.tensor_tensor(F[:], N2[:], Xr[:], ALU.mult)
            nc.vector.tensor_tensor(F[:], F[:], RD[:], ALU.mult)
            nc.vector.tensor_tensor(
                out_ap.rearrange("p (j k) -> p j k", j=10),
                SP[:].rearrange("p (j k) -> p j k", j=10),
                F[:].unsqueeze(2).broadcast_to([1, 10, 16]), ALU.mult)

        def agreement(UH, VT, BL, first):
            import dataclasses
            VBS = rtp.tile([128, 160], f32, tag="vbs", name="VBS")
            vsrc = dataclasses.replace(
                VT[:], ap=[[VT[:].ap[0][0], 1], [0, 128], [1, 160]])
            nc.sync.dma_start(out=VBS[:], in_=vsrc)
            for ic in range(9):
                PRD = rtp.tile([128, 10, 16], f32, tag="prd")
                nc.gpsimd.tensor_tensor(
                    PRD[:], UH[:, ic, :].rearrange("p (j k) -> p j k", j=10),
                    VBS[:].rearrange("p (j k) -> p j k", j=10), ALU.mult)
                if first:
                    nc.vector.tensor_reduce(BL[:, ic, :], PRD[:], AX.X, ALU.add)
                else:
                    DL = rtp.tile([128, 10], f32, tag="dl")
                    nc.vector.tensor_reduce(DL[:], PRD[:], AX.X, ALU.add)
                    nc.vector.tensor_tensor(BL[:, ic, :], BL[:, ic, :], DL[:],
                                            ALU.add)

        def route_one_image(b):
            o, bl = b // OCT, b % OCT
            UH = uhp.tile([128, 9, 160], f32)
            for ic in range(9):
                nc.vector.tensor_scalar(UH[:, ic, :], WR[:, ic, 0, :],
                                        UT[o][ic][:, 0, bl:bl + 1], None, ALU.mult)
                for v in range(1, 8):
                    nc.vector.scalar_tensor_tensor(
                        UH[:, ic, :], WR[:, ic, v, :], UT[o][ic][:, v, bl:bl + 1],
                        UH[:, ic, :], ALU.mult, ALU.add)
            # iter 0: c = 1/10 uniform (stream a bf16 shadow of u_hat)
            UHB = uhp.tile([128, 9, 160], bf16, tag="uhb", name="UHB")
            nc.vector.tensor_copy(UHB[:], UH[:])
            SP = rps.tile([1, 160], f32, tag="s", bufs=2, name="SP")
            for ic in range(9):
                nc.tensor.matmul(SP[:], tenth[:], UHB[:, ic, :],
                                 start=(ic == 0), stop=(ic == 8))
            VT = rtp.tile([1, 160], f32, tag="vt")
            squash(SP, VT[:])
            BL = rtp.tile([128, 9, 10], f32, tag="bl")
            agreement(UH, VT, BL, first=True)
            # iters 1, 2
            for it in (1, 2):
                E = rtp.tile([128, 9, 10], f32, tag="e")
                nc.scalar.activation(E[:], BL[:], AF.Exp)
                SM = rtp.tile([128, 9], f32, tag="sm")
                nc.vector.tensor_reduce(SM[:], E[:], AX.X, ALU.add)
                RS = rtp.tile([128, 9], f32, tag="rs")
                nc.vector.reciprocal(RS[:], SM[:])
                EN = rtp.tile([128, 9, 10], f32, tag="en")
                nc.vector.tensor_tensor(
                    EN[:], E[:], RS[:].unsqueeze(2).broadcast_to([128, 9, 10]),
                    ALU.mult)
                SP2 = rps.tile([1, 160], f32, tag="s", bufs=2, name="SP2")
                for ic in range(9):
                    T = rtp.tile([128, 10, 16], bf16, tag="t")
                    nc.gpsimd.tensor_tensor(
                        T[:], UH[:, ic, :].rearrange("p (j k) -> p j k", j=10),
                        EN[:, ic, :].unsqueeze(2).broadcast_to([128, 10, 16]),
                        ALU.mult)
                    nc.tensor.matmul(SP2[:], ones_c[:],
                                     T[:].rearrange("p j k -> p (j k)"),
                                     start=(ic == 0), stop=(ic == 8))
                VT2 = rtp.tile([1, 160], f32, tag="vt")
                squash(SP2, VT2[:])
                if it == 1:
                    agreement(UH, VT2, BL, first=False)
                else:
                    # engines cannot shift partitions; route via DMA
                    nc.sync.dma_start(out=V_ALL[b:b + 1, :], in_=VT2[:])

        # ---------------- emit: software-pipelined over octets ----------------
        for o in range(n_oct):
            A3 = [a3p.tile([128, OCT, 20, 20], bf16, tag=f"a3_{h}", name=f"a3_{h}")
                  for h in range(2)]
            for bl in range(OCT):
                b = o * OCT + bl
                X3 = convs_one_image(b)
                conv3_one_image(X3, A3, bl)
                if o > 0:
                    route_one_image((o - 1) * OCT + bl)
            prim_and_shuffle(o, A3)
        cctx.close()
        for bl in range(OCT):
            route_one_image((n_oct - 1) * OCT + bl)
        rctx.close()

        # ---------------- mask + decoder ----------------
        decp = ctx.enter_context(tc.tile_pool(name="decp", bufs=1))
        dps = ctx.enter_context(tc.tile_pool(name="dps", bufs=1, space="PSUM"))
        Yt = decp.tile([b_loc, 1], i32)
        nc.sync.dma_start(out=Yt[:], in_=y_d.ap().unsqueeze(1))
        Yf = decp.tile([b_loc, 1], f32)
        nc.vector.tensor_copy(Yf[:], Yt[:])
        JI = decp.tile([b_loc, 160], i32)
        nc.gpsimd.iota(JI[:].rearrange("p (j k) -> p j k", j=10),
                       [[1, 10], [0, 16]], base=0, channel_multiplier=0)
        JF = decp.tile([b_loc, 160], f32)
        nc.vector.tensor_copy(JF[:], JI[:])
        MK = decp.tile([b_loc, 160], f32)
        nc.vector.tensor_tensor(MK[:], JF[:],
                                Yf[:].broadcast_to([b_loc, 160]), ALU.is_equal)
        VM = decp.tile([b_loc, 160], f32)
        nc.vector.tensor_tensor(VM[:], V_ALL[:], MK[:], ALU.mult)
        nc.sync.dma_start(out=v_d.ap(), in_=V_ALL[:])

        # vmT via two PE transposes
        TA = dps.tile([128, b_loc], f32, tag="tp2", name="TA")
        nc.tensor.transpose(TA[:], VM[:, 0:128], ident[0:b_loc, 0:b_loc])
        VTA = decp.tile([128, b_loc], bf16)
        nc.scalar.activation(VTA[:], TA[:], AF.Copy)
        TBp = dps.tile([32, b_loc], f32, tag="tp2", name="TBp")
        nc.tensor.transpose(TBp[:], VM[:, 128:160], ident[0:b_loc, 0:b_loc])
        VTB = decp.tile([32, b_loc], bf16)
        nc.scalar.activation(VTB[:], TBp[:], AF.Copy)

        D1A = decp.tile([128, 1024], bf16)
        nc.sync.dma_start(out=D1A[:], in_=d1w_d.ap()[0:128, :])
        D1Bw = decp.tile([32, 1024], bf16)
        nc.sync.dma_start(out=D1Bw[:], in_=d1w_d.ap()[128:160, :])
        D1b = decp.tile([128, 8], f32)
        nc.sync.dma_start(out=D1b[:], in_=d1b_d.ap().rearrange("(m p) -> p m", m=8))
        D2W = decp.tile([128, 8, 2048], bf16)
        nc.sync.dma_start(out=D2W[:],
                          in_=d2w_d.ap().rearrange("(c p) m -> p c m", p=128))
        D2b = decp.tile([128, 16], f32)
        nc.sync.dma_start(out=D2b[:], in_=d2b_d.ap().rearrange("(m p) -> p m", m=16))
        D3W = decp.tile([128, 16, 1600], bf16)
        nc.sync.dma_start(out=D3W[:],
                          in_=d3w_d.ap().rearrange("(c p) m -> p c m", p=128))
        D3b = decp.tile([128, 13], f32)
        nc.sync.dma_start(out=D3b[:, 0:12],
                          in_=d3b_d.ap()[0:1536].rearrange("(m p) -> p m", m=12))
        nc.sync.dma_start(out=D3b[0:64, 12:13], in_=d3b_d.ap()[1536:1600].unsqueeze(1))

        R1 = decp.tile([128, 8, b_loc], bf16)
        for mc in range(8):
            PS = dps.tile([128, b_loc], f32, tag="d", bufs=2, name="PS")
            nc.tensor.matmul(PS[:], D1A[:, 128 * mc:128 * mc + 128], VTA[:],
                             start=True, stop=False)
            nc.tensor.matmul(PS[:], D1Bw[:, 128 * mc:128 * mc + 128], VTB[:],
                             start=False, stop=True)
            nc.scalar.activation(R1[:, mc, :], PS[:], AF.Relu,
                                 bias=D1b[:, mc:mc + 1])
        R2 = decp.tile([128, 16, b_loc], bf16)
        for mc in range(16):
            PS = dps.tile([128, b_loc], f32, tag="d", bufs=2, name="PS")
            for kc in range(8):
                nc.tensor.matmul(PS[:], D2W[:, kc, 128 * mc:128 * mc + 128],
                                 R1[:, kc, :], start=(kc == 0), stop=(kc == 7))
            nc.scalar.activation(R2[:, mc, :], PS[:], AF.Relu,
                                 bias=D2b[:, mc:mc + 1])
        for mc in range(13):
            mw = 128 if mc < 12 else 64
            PS = dps.tile([128, b_loc], f32, tag="d", bufs=2, name="PS")
            for kc in range(16):
                nc.tensor.matmul(PS[0:mw, :],
                                 D3W[:, kc, 128 * mc:128 * mc + mw],
                                 R2[:, kc, :], start=(kc == 0), stop=(kc == 15))
            R3 = decp.tile([128, b_loc], f32, tag="r3")
            nc.scalar.activation(R3[0:mw, :], PS[0:mw, :], AF.Sigmoid,
                                 bias=D3b[0:mw, mc:mc + 1])
            nc.sync.dma_start(
                out=rec_d.ap()[:, 128 * mc:128 * mc + mw].transpose([1, 0]),
                in_=R3[0:mw, :])

    nc.compile()
    return nc


def prep_inputs(inputs, lo, hi):
    """Host-side prep: shard batch, cast/relayout weights. Returns in_map."""
    g = {k: np.asarray(v) for k, v in inputs.items()}
    bf = ml_dtypes.bfloat16

    w2 = g["conv2_k"].astype(np.float32)          # [9,9,64,128]
    w2p = np.zeros((45, 128, 128), np.float32)
    for kw in range(9):
        for pr in range(5):
            t = kw * 5 + pr
            kh = 2 * pr
            w2p[t, 0:64] = w2[kh, kw]
            if kh + 1 <= 8:
                w2p[t, 64:128] = w2[kh + 1, kw]
    w3 = g["conv3_k"].astype(np.float32)          # [9,9,128,256]
    w3t = np.zeros((162, 128, 128), np.float32)
    for kh in range(9):
        for kw in range(9):
            for h in range(2):
                w3t[(kh * 9 + kw) * 2 + h] = w3[kh, kw, :, 128 * h:128 * h + 128]
    wp = g["prim_k"].astype(np.float32)           # [9,9,256,256]
    # [h, tap_pair, ci_part, (tap%2, cin... )]: device reads [h, t2, p, s, co]
    # where lhsT for (tap, cin, h) = wpt2[h, tap//2, :, tap%2 ... ] -- we pack
    # (tap%2, cin) into the s axis of size 2x2=4? No: tile is [128, 2, 2, 128]
    # (tap-parity, cin). DMA src must match: [h, tap//2, p, (parity, cin)*? ]
    wpt4 = np.zeros((2, 41, 128, 2, 2, 128), np.float32)  # h, t2, p, parity, cin, co
    for kh in range(9):
        for kw in range(9):
            tap = kh * 9 + kw
            for cin in range(2):
                for h in range(2):
                    wpt4[h, tap // 2, :, tap % 2, cin, :] = \
                        wp[kh, kw, 128 * cin:128 * cin + 128, 128 * h:128 * h + 128]
    wr = g["w"].astype(np.float32)                # [1152,10,16,8]
    wrt = wr.transpose(0, 3, 1, 2).reshape(9, 128, 8, 160)

    return dict(
        x=np.ascontiguousarray(g["x"][lo:hi, :, :, 0], dtype=np.float32),
        y=np.ascontiguousarray(g["y"][lo:hi], dtype=np.int32),
        w1=np.ascontiguousarray(g["conv1_k"].reshape(25, 64), dtype=np.float32),
        b1=g["conv1_b"].astype(np.float32),
        w2=np.ascontiguousarray(w2p.transpose(1, 0, 2)).astype(bf),
        b2=g["conv2_b"].astype(np.float32),
        w3=np.ascontiguousarray(w3t.transpose(1, 0, 2)).astype(bf),
        b3=g["conv3_b"].astype(np.float32),
        wp=wpt4.astype(bf),
        bp=g["prim_b"].astype(np.float32),
        wr=np.ascontiguousarray(wrt.transpose(1, 0, 2, 3)).astype(bf),
        d1w=g["d1_w"].astype(bf), d1b=g["d1_b"].astype(np.float32),
        d2w=g["d2_w"].astype(bf), d2b=g["d2_b"].astype(np.float32),
        d3w=g["d3_w"].astype(bf), d3b=g["d3_b"].astype(np.float32),
    )


def kernel(**inputs):
    if "nc" not in _cache:
        _cache["nc"] = build_program()
    nc = _cache["nc"]
    in_maps = [prep_inputs(inputs, c * B_LOC, (c + 1) * B_LOC)
               for c in range(N_CORES)]
    try:
        res = run_bass_kernel_spmd(nc, in_maps, list(range(N_CORES)))
    except Exception:
        # transient NRT device wedges recover on retry
        import time as _time
        _time.sleep(2.0)
        res = run_bass_kernel_spmd(nc, in_maps, list(range(N_CORES)))
    v = np.concatenate([res.results[c]["v_out"].reshape(B_LOC, 1, 10, 16)
                        for c in range(N_CORES)], axis=0)
    recon = np.concatenate([res.results[c]["recon"] for c in range(N_CORES)],
                           axis=0)
    return v.astype(np.float32), recon.astype(np.float32)
